# revision 63
# baseline (speedup 1.0000x reference)
"""ALayer kernel for 8 TRN2 NeuronCores — pure data parallel over batch.

Per-core shard: 4 images of [256, 56, 56].
  h  = relu(conv3x3(x_in, w1))      # 256 -> 16 ch
  A  = sigmoid(conv3x3(h, w2))      # 16 -> 1 ch
  out = x_out * box3x3(A)           # broadcast over 256 ch

v20 design — no DMA in the mid-pipeline dep chains (TRN2 SDMA serves
SBUF->SBUF SWDGE transfers only when bulk HBM queues are empty, so any
DMA-built intermediate starves; measured 7-20us stalls):
  conv1: v6's column-tiled rounds (4 concurrent 32-col strips, fp8,
         M=16, N=392; 36 rounds/image).  Relu evacs write the h plane
         = hcol[0:16] rows 1..58 of a 61-row guard-padded tile.
  conv2: K=48 dy-fold: hcol[16*dy+m] = plane shifted 58*dy (2 paced
         SWDGE copies, images 0-2; dx shifts come free via the rhs AP).
         3 col-tiled rounds x 2 supergroups, OVERLAPPING 9-row blocks
         (psum 504 <= 512).  Image 3 uses the direct 9-tap form (18
         rounds, zero DMA deps) so the tail has no DMA latency at all.
  box:   box3x3 = rowsum3+colsum3 computed per-partition on the Pool
         engine from the 9-row sigmoid blocks (each block carries its
         halo rows, so no cross-partition traffic), then broadcast to
         128 partitions with a K=128 selector matmul; evac to bf16 ab
         (ACT/DVE), bf16 muls (DVE 2x) in <=1us chunks, stores on the
         ACT HWDGE ring.
  Loads ride the SP HWDGE ring, paced by pool-reuse WAR gates
         (xpad/xo bufs=2) + emission order so the ring has idle windows
         exactly when the hcol copies fire.  Zero-padded-K matmuls keep
         the HAM activity monitor seeing a busy array (clock 8/8).
"""

import numpy as np
import ml_dtypes

import concourse.bass as bass
import concourse.tile as tile
import concourse.mybir as mybir
from concourse import bacc
from concourse.bass_utils import run_bass_kernel_spmd

BF16 = mybir.dt.bfloat16
FP8 = mybir.dt.float8e4
F32 = mybir.dt.float32

B, C, H, W = 32, 256, 56, 56
NCORES = 8
BL = B // NCORES          # images per core
KCH = 2                   # 256 = 2 chunks of 128
HP = H + 2                # padded plane side (58)
HT = 61                   # hcol tile rows: top guard + plane + bottom slack
HW = H * W                # 3136
PL = HP * HP              # 3364
PL2 = HT * HP             # 3538

_cache = {}


def _build():
    nc = bacc.Bacc("TRN2", target_bir_lowering=False, debug=False)

    xin_d = nc.dram_tensor("xin", [BL, KCH, 128, PL], FP8, kind="ExternalInput").ap()
    xout_d = nc.dram_tensor("xout", [BL, 128, KCH, HW], BF16, kind="ExternalInput").ap()
    w1_d = nc.dram_tensor("w1t", [128, KCH, 9, 16], FP8, kind="ExternalInput").ap()
    w2_d = nc.dram_tensor("w2t", [128, 12], BF16, kind="ExternalInput").ap()
    out_d = nc.dram_tensor("out", [BL, 128, KCH, HW], BF16, kind="ExternalOutput").ap()

    with tile.TileContext(nc) as tc:
        with (
            tc.tile_pool(name="const", bufs=1) as constp,
            tc.tile_pool(name="xpad", bufs=4) as xpadp,
            tc.tile_pool(name="hcol", bufs=2) as hcolp,
            tc.tile_pool(name="at9", bufs=2) as atp,
            tc.tile_pool(name="sums", bufs=2) as sump,
            tc.tile_pool(name="tmps", bufs=4) as tmpp,
            tc.tile_pool(name="ab", bufs=2) as abp,
            tc.tile_pool(name="xo", bufs=4) as xop,
            tc.tile_pool(name="ot", bufs=2) as otp,
            tc.tile_pool(name="ps_h", bufs=2, space="PSUM") as ps_h,
            tc.tile_pool(name="ps_a", bufs=2, space="PSUM") as ps_a,
            tc.tile_pool(name="ps_b", bufs=3, space="PSUM") as ps_b,
            tc.tile_pool(name="ps_w", bufs=1, space="PSUM") as ps_w,
        ):
            w1sb = constp.tile([128, KCH, 9, 16], FP8)
            w2sb = constp.tile([128, 12], BF16)
            sel = constp.tile([128, 4, 128], BF16)
            wl = constp.tile([128, 128], FP8)
            wr = constp.tile([128, 512], FP8)

            xpads = [
                xpadp.tile([128, KCH, HP, HP], FP8, name="xpad")
                for _ in range(BL)
            ]
            xos = [xop.tile([128, KCH, HW], BF16, name="xo") for _ in range(BL)]
            hcols = [
                hcolp.tile([128, HT, HP], BF16, name="hcol") for _ in range(BL)
            ]
            at9s = [
                atp.tile([128, 2, 9, HP], BF16, name="at9") for _ in range(BL)
            ]
            rsums = [
                sump.tile([128, 2, 9, 56], BF16, name="rsum") for _ in range(BL)
            ]
            bbs = [
                sump.tile([128, 2, 7, 56], BF16, name="bb") for _ in range(BL)
            ]
            MID = 30 * HP

            def load_xin(img, split):
                xpf = xpads[img].rearrange("p k r w -> p k (r w)")
                if split:
                    for k in range(KCH):
                        nc.sync.dma_start(xpf[:, k, 0:MID], xin_d[img, k, :, 0:MID])
                    for k in range(KCH):
                        nc.sync.dma_start(xpf[:, k, MID:PL], xin_d[img, k, :, MID:PL])
                else:
                    for k in range(KCH):
                        nc.sync.dma_start(xpf[:, k, :], xin_d[img, k, :, :])

            # ---- loads: head batch, then WAR-gated stragglers ----
            xpf0 = xpads[0].rearrange("p k r w -> p k (r w)")
            nc.sync.dma_start(xpf0[:, 0, 0:MID], xin_d[0, 0, :, 0:MID])
            nc.sync.dma_start(w1sb[:], w1_d[:])
            nc.sync.dma_start(w2sb[:], w2_d[:])
            nc.sync.dma_start(xpf0[:, 1, 0:MID], xin_d[0, 1, :, 0:MID])
            nc.sync.dma_start(xpf0[:, 0, MID:PL], xin_d[0, 0, :, MID:PL])
            nc.sync.dma_start(xpf0[:, 1, MID:PL], xin_d[0, 1, :, MID:PL])
            load_xin(1, True)
            nc.sync.dma_start(xos[0][:], xout_d[0])
            load_xin(2, False)
            nc.sync.dma_start(xos[1][:], xout_d[1])
            load_xin(3, False)
            nc.sync.dma_start(xos[2][:], xout_d[2])
            nc.sync.dma_start(xos[3][:], xout_d[3])

            # ---- constants; zero guard planes for both pool buffers ----
            nc.vector.memset(sel[:], 0.0)
            for j in range(4):
                nc.vector.memset(sel[32 * j : 32 * j + 1, j, :], 1.0)
            nc.gpsimd.memset(wl[:], 0.0)
            nc.gpsimd.memset(wr[:], 0.0)
            for img in range(2):
                nc.scalar.memzero(hcols[img][:, :, :])
                nc.vector.memset(at9s[img][:, :, :, 0], 0.0)
                nc.vector.memset(at9s[img][:, :, :, 57], 0.0)

            def warm(n):
                for _ in range(n):
                    wp = ps_w.tile([128, 512], F32)
                    nc.tensor.matmul(
                        wp[:], wl[:], wr[:],
                        start=True, stop=True, skip_group_check=True,
                    )

            def emit_hcol(img):
                """2 independent SWDGE copies: dy=1,2 row-shifted replicas."""
                hf = hcols[img].rearrange("p r w -> p (r w)")
                nc.gpsimd.dma_start(hf[16:32, 0 : PL2 - 58], hf[0:16, 58:PL2])
                nc.gpsimd.dma_start(hf[32:48, 0 : PL2 - 116], hf[0:16, 116:PL2])

            def gen_conv1(img):
                """36 PE rounds; relu evacs into plane rows 1..58."""
                xpad = xpads[img]
                h1 = hcols[img]
                for s in range(2):
                    ps = ps_h.tile([128, 7, 56], F32)
                    rnd = 0
                    for k in range(KCH):
                        for t in range(9):
                            dy, dx = t // 3, t % 3
                            for j in range(4):
                                rs = 28 * s + j + dy
                                nc.tensor.matmul(
                                    ps[32 * j : 32 * j + 16],
                                    w1sb[:, k, t, :],
                                    xpad[:, k, rs : rs + 25 : 4, dx : dx + 56],
                                    start=(rnd == 0),
                                    stop=(rnd == 17),
                                    tile_position=(0, 32 * j),
                                    skip_group_check=True,
                                )
                            rnd += 1
                            if rnd == 18:
                                for j in range(4):
                                    r0 = 2 + 28 * s + j
                                    dst = h1[0:16, r0 : r0 + 25 : 4, 1:57]
                                    if j < 2:
                                        nc.scalar.activation(
                                            dst, ps[32 * j : 32 * j + 16],
                                            mybir.ActivationFunctionType.Relu,
                                        )
                                    else:
                                        nc.vector.tensor_scalar_max(
                                            dst, ps[32 * j : 32 * j + 16], 0.0
                                        )
                                if s == 1:
                                    # dy=1,2 row-shifted replicas for the
                                    # K=48 conv2 fold — DVE tensor_copy
                                    # runs in 4x bf16 mode (~0.9us each),
                                    # no DMA anywhere in the chain
                                    hf = h1.rearrange("p r w -> p (r w)")
                                    nc.vector.tensor_copy(
                                        hf[32:48, 0 : PL2 - 58],
                                        hf[0:16, 58:PL2],
                                    )
                                    nc.vector.tensor_copy(
                                        hf[64:80, 0 : PL2 - 116],
                                        hf[0:16, 116:PL2],
                                    )
                            yield

            def emit_boxsums(img, s):
                """Pool-engine rowsum3+colsum3 within the 9-row blocks."""
                at9, rsum, bb = at9s[img], rsums[img], bbs[img]
                if s == 0:
                    nc.gpsimd.memset(at9[0:1, 0, 0, :], 0.0)   # yp=0 pad row
                else:
                    nc.gpsimd.memset(at9[96:97, 1, 8, :], 0.0)  # yp=57 pad row
                # rowsum3 on Pool (it has headroom), colsum3 on DVE (2x)
                tmp9 = tmpp.tile([128, 9, 56], BF16, name="tmp9")
                nc.gpsimd.tensor_add(
                    tmp9[:], at9[:, s, :, 0:56], at9[:, s, :, 1:57]
                )
                nc.gpsimd.tensor_add(
                    rsum[:, s], tmp9[:], at9[:, s, :, 2:58]
                )
                tmp7 = tmpp.tile([128, 7, 56], BF16, name="tmp7")
                nc.vector.tensor_add(
                    tmp7[:], rsum[:, s, 0:7], rsum[:, s, 1:8]
                )
                nc.vector.tensor_add(bb[:, s], tmp7[:], rsum[:, s, 2:9])

            def gen_conv2(img):
                """6 PE rounds (K=48 dy-fold); sigmoid; Pool box sums."""
                hcol = hcols[img]
                at9 = at9s[img]
                for s in range(2):
                    ps = ps_a.tile([128, 9, 56], F32)
                    for dx in range(3):
                        for j in range(4):
                            b = 4 * s + j
                            nc.tensor.matmul(
                                ps[32 * j : 32 * j + 1],
                                w2sb[:, dx : dx + 1],
                                hcol[:, 7 * b : 7 * b + 9, dx : dx + 56],
                                start=(dx == 0), stop=(dx == 2),
                                tile_position=(0, 32 * j),
                                skip_group_check=True,
                            )
                        if dx < 2:
                            yield
                    nc.scalar.activation(
                        at9[:, s, :, 1:57], ps[:],
                        mybir.ActivationFunctionType.Sigmoid,
                    )
                    emit_boxsums(img, s)
                    yield

            def gen_conv2_direct(img):
                """18 direct 9-tap rounds — no hcol copies, no DMA deps.
                Used for the last image so the tail chain is DMA-free."""
                hcol = hcols[img]
                at9 = at9s[img]
                for s in range(2):
                    ps = ps_a.tile([128, 9, 56], F32)
                    rnd = 0
                    for dy in range(3):
                        for dx in range(3):
                            for j in range(4):
                                b = 4 * s + j
                                nc.tensor.matmul(
                                    ps[32 * j : 32 * j + 1],
                                    w2sb[:, 3 + rnd : 4 + rnd],
                                    hcol[:, 7 * b + dy : 7 * b + dy + 9,
                                         dx : dx + 56],
                                    start=(rnd == 0), stop=(rnd == 8),
                                    tile_position=(0, 32 * j),
                                    skip_group_check=True,
                                )
                            rnd += 1
                            if rnd == 9:
                                nc.scalar.activation(
                                    at9[:, s, :, 1:57], ps[:],
                                    mybir.ActivationFunctionType.Sigmoid,
                                )
                                emit_boxsums(img, s)
                            yield

            def gen_box(img):
                """8 blocks: selector-matmul broadcast, evac, bf16 muls."""
                bb = bbs[img]
                xo = xos[img]
                ab = abp.tile([128, 56, 56], BF16)
                abf = ab.rearrange("p r w -> p (r w)")
                ot = otp.tile([128, KCH, HW], BF16)

                def halfdone(h):
                    s0, s1 = (0, 1568) if h == 0 else (1568, HW)
                    for k in range(KCH):
                        nc.vector.tensor_mul(
                            ot[:, k, s0:s1], xo[:, k, s0:s1], abf[:, s0:s1]
                        )
                    nc.scalar.dma_start(
                        out_d[img, :, :, s0:s1], ot[:, :, s0:s1]
                    )

                for b in range(8):
                    s, j = b // 4, b % 4
                    psb = ps_b.tile([128, 7, 56], F32)
                    nc.tensor.matmul(
                        psb[:], sel[:, j, :], bb[:, s, :, :],
                        start=True, stop=True,
                    )
                    dst = ab[:, 7 * b : 7 * b + 7, :]
                    if b % 2 == 0:
                        nc.scalar.activation(
                            dst, psb[:], mybir.ActivationFunctionType.Copy
                        )
                    else:
                        nc.vector.tensor_copy(dst, psb[:])
                    if b == 3:
                        halfdone(0)
                    yield
                halfdone(1)
                yield

            def run(gen, n):
                for _ in range(n):
                    next(gen, None)

            c1 = [gen_conv1(i) for i in range(BL)]
            c2 = [gen_conv2(i) for i in range(BL)]
            bx = [gen_box(i) for i in range(BL)]

            def block(i):
                # c1 r0-23 solo, conv2(i-1) 6 rounds at r24-29 (1:1),
                # box(i-2) at r30-35 (1:1) + small burst.
                if i >= 3:
                    run(bx[i - 3], 1)    # deferred second-half muls+store
                run(c1[i], 24)
                for _ in range(6):
                    run(c2[i - 1], 1)
                    run(c1[i], 1)
                for _ in range(6):
                    run(bx[i - 2], 1)
                    run(c1[i], 1)
                run(bx[i - 2], 2)

            warm(3)
            run(c1[0], 36)
            run(c1[1], 24)
            for _ in range(6):
                run(c2[0], 1)
                run(c1[1], 1)
            run(c1[1], 6)
            block(2)
            block(3)
            # tail: finish box(1); box(2) first (its bb is long ready),
            # then conv2(3) once the DVE replicas land, then box(3)
            run(bx[1], 1)
            run(bx[2], 4)
            for _ in range(3):
                run(c2[3], 2)
                run(bx[2], 1)
            run(bx[2], 1)
            warm(8)
            run(bx[3], 4)
            warm(2)
            run(bx[3], 4)
            run(bx[2], 1)
            run(bx[3], 1)

    nc.compile()
    return nc


def _prep_shards(x_in, x_out, w1, w2):
    bf16 = ml_dtypes.bfloat16
    fp8 = ml_dtypes.float8_e4m3
    # w1t[c, k, t, m] = w1[m, 128k + c, dy, dx],  t = 3*dy + dx
    w1t = np.ascontiguousarray(
        w1.reshape(16, KCH, 128, 9).transpose(2, 1, 3, 0)
    ).astype(fp8)
    # w2t cols 0-2:  dy-fold (replicas at 32-aligned partition bases)
    #   w2t[32*dy + m, dx] = w2[0, m, dy, dx]
    # w2t cols 3-11: direct taps  w2t[m, 3 + 3*dy + dx] = w2[0, m, dy, dx]
    w2t = np.zeros((128, 12), dtype=bf16)
    for dy in range(3):
        w2t[32 * dy : 32 * dy + 16, 0:3] = w2[0, :, dy, :].astype(bf16)
    w2t[0:16, 3:12] = w2[0].reshape(16, 9).astype(bf16)
    xi = np.zeros((NCORES, BL, KCH, 128, HP, HP), dtype=fp8)
    xi[..., 1 : 1 + H, 1 : 1 + W] = (
        x_in.reshape(NCORES, BL, KCH, 128, H, W).astype(fp8)
    )
    xi = xi.reshape(NCORES, BL, KCH, 128, PL)
    # xout[img, c_partition, k, hw]
    xo = np.ascontiguousarray(
        x_out.reshape(NCORES, BL, KCH, 128, HW).transpose(0, 1, 3, 2, 4)
    ).astype(bf16)
    return [
        {
            "xin": np.ascontiguousarray(xi[i]),
            "xout": xo[i],
            "w1t": w1t,
            "w2t": w2t,
        }
        for i in range(NCORES)
    ]


def _run(in_maps, trace=False):
    if "nc" not in _cache:
        _cache["nc"] = _build()
    return run_bass_kernel_spmd(
        _cache["nc"], in_maps, core_ids=list(range(NCORES)), trace=trace
    )


def kernel(x_in, x_out, w1, w2, _trace=False):
    in_maps = _prep_shards(
        np.asarray(x_in, dtype=np.float32),
        np.asarray(x_out, dtype=np.float32),
        np.asarray(w1, dtype=np.float32),
        np.asarray(w2, dtype=np.float32),
    )
    res = _run(in_maps, trace=_trace)
    # out[img, c_partition, k, hw] bf16 -> [B, C, H, W] fp32
    out = np.stack([res.results[i]["out"] for i in range(NCORES)])
    kernel.last_exec_time_ns = res.exec_time_ns
    out = out.astype(np.float32).transpose(0, 1, 3, 2, 4)
    return out.reshape(B, C, H, W)


# revision 65
# speedup vs baseline: 1.0531x; 1.0531x over previous
"""ALayer kernel for 8 TRN2 NeuronCores — pure data parallel over batch.

Per-core shard: 4 images of [256, 56, 56].
  h  = relu(conv3x3(x_in, w1))      # 256 -> 16 ch
  A  = sigmoid(conv3x3(h, w2))      # 16 -> 1 ch
  out = x_out * box3x3(A)           # broadcast over 256 ch

v20 design — no DMA in the mid-pipeline dep chains (TRN2 SDMA serves
SBUF->SBUF SWDGE transfers only when bulk HBM queues are empty, so any
DMA-built intermediate starves; measured 7-20us stalls):
  conv1: v6's column-tiled rounds (4 concurrent 32-col strips, fp8,
         M=16, N=392; 36 rounds/image).  Relu evacs write the h plane
         = hcol[0:16] rows 1..58 of a 61-row guard-padded tile.
  conv2: K=48 dy-fold: hcol[16*dy+m] = plane shifted 58*dy (2 paced
         SWDGE copies, images 0-2; dx shifts come free via the rhs AP).
         3 col-tiled rounds x 2 supergroups, OVERLAPPING 9-row blocks
         (psum 504 <= 512).  Image 3 uses the direct 9-tap form (18
         rounds, zero DMA deps) so the tail has no DMA latency at all.
  box:   box3x3 = rowsum3+colsum3 computed per-partition on the Pool
         engine from the 9-row sigmoid blocks (each block carries its
         halo rows, so no cross-partition traffic), then broadcast to
         128 partitions with a K=128 selector matmul; evac to bf16 ab
         (ACT/DVE), bf16 muls (DVE 2x) in <=1us chunks, stores on the
         ACT HWDGE ring.
  Loads ride the SP HWDGE ring, paced by pool-reuse WAR gates
         (xpad/xo bufs=2) + emission order so the ring has idle windows
         exactly when the hcol copies fire.  Zero-padded-K matmuls keep
         the HAM activity monitor seeing a busy array (clock 8/8).
"""

import numpy as np
import ml_dtypes

import concourse.bass as bass
import concourse.tile as tile
import concourse.mybir as mybir
from concourse import bacc
from concourse.bass_utils import run_bass_kernel_spmd

BF16 = mybir.dt.bfloat16
FP8 = mybir.dt.float8e4
F32 = mybir.dt.float32

B, C, H, W = 32, 256, 56, 56
NCORES = 8
BL = B // NCORES          # images per core
KCH = 2                   # 256 = 2 chunks of 128
HP = H + 2                # padded plane side (58)
HT = 61                   # hcol tile rows: top guard + plane + bottom slack
HW = H * W                # 3136
PL = HP * HP              # 3364
PL2 = HT * HP             # 3538

_cache = {}


def _build():
    nc = bacc.Bacc("TRN2", target_bir_lowering=False, debug=False)

    xin_d = nc.dram_tensor("xin", [BL, KCH, 128, PL], FP8, kind="ExternalInput").ap()
    xout_d = nc.dram_tensor("xout", [BL, 128, KCH, HW], BF16, kind="ExternalInput").ap()
    w1_d = nc.dram_tensor("w1t", [128, KCH, 9, 16], FP8, kind="ExternalInput").ap()
    w2_d = nc.dram_tensor("w2t", [128, 12], BF16, kind="ExternalInput").ap()
    out_d = nc.dram_tensor("out", [BL, 128, KCH, HW], BF16, kind="ExternalOutput").ap()

    with tile.TileContext(nc) as tc:
        with (
            tc.tile_pool(name="const", bufs=1) as constp,
            tc.tile_pool(name="xpad", bufs=4) as xpadp,
            tc.tile_pool(name="hcol", bufs=2) as hcolp,
            tc.tile_pool(name="at9", bufs=2) as atp,
            tc.tile_pool(name="sums", bufs=2) as sump,
            tc.tile_pool(name="tmps", bufs=4) as tmpp,
            tc.tile_pool(name="ab", bufs=2) as abp,
            tc.tile_pool(name="xo", bufs=4) as xop,
            tc.tile_pool(name="ot", bufs=2) as otp,
            tc.tile_pool(name="ps_h", bufs=2, space="PSUM") as ps_h,
            tc.tile_pool(name="ps_a", bufs=2, space="PSUM") as ps_a,
            tc.tile_pool(name="ps_b", bufs=3, space="PSUM") as ps_b,
            tc.tile_pool(name="ps_w", bufs=1, space="PSUM") as ps_w,
        ):
            w1sb = constp.tile([128, KCH, 9, 16], FP8)
            w2sb = constp.tile([128, 12], BF16)
            sel = constp.tile([128, 4, 128], BF16)
            wl = constp.tile([128, 128], FP8)
            wr = constp.tile([128, 512], FP8)

            xpads = [
                xpadp.tile([128, KCH, HP, HP], FP8, name="xpad")
                for _ in range(BL)
            ]
            xos = [xop.tile([128, KCH, HW], BF16, name="xo") for _ in range(BL)]
            hcols = [
                hcolp.tile([128, HT, HP], BF16, name="hcol") for _ in range(BL)
            ]
            at9s = [
                atp.tile([128, 2, 9, HP], BF16, name="at9") for _ in range(BL)
            ]
            rsums = [
                sump.tile([128, 2, 9, 56], BF16, name="rsum") for _ in range(BL)
            ]
            bbs = [
                sump.tile([128, 2, 7, 56], BF16, name="bb") for _ in range(BL)
            ]
            MID = 30 * HP

            def load_xin(img, split):
                xpf = xpads[img].rearrange("p k r w -> p k (r w)")
                if split:
                    for k in range(KCH):
                        nc.sync.dma_start(xpf[:, k, 0:MID], xin_d[img, k, :, 0:MID])
                    for k in range(KCH):
                        nc.sync.dma_start(xpf[:, k, MID:PL], xin_d[img, k, :, MID:PL])
                else:
                    for k in range(KCH):
                        nc.sync.dma_start(xpf[:, k, :], xin_d[img, k, :, :])

            # ---- loads: head batch, then WAR-gated stragglers ----
            xpf0 = xpads[0].rearrange("p k r w -> p k (r w)")
            nc.sync.dma_start(xpf0[:, 0, 0:MID], xin_d[0, 0, :, 0:MID])
            nc.sync.dma_start(w1sb[:], w1_d[:])
            nc.sync.dma_start(w2sb[:], w2_d[:])
            nc.sync.dma_start(xpf0[:, 1, 0:MID], xin_d[0, 1, :, 0:MID])
            nc.sync.dma_start(xpf0[:, 0, MID:PL], xin_d[0, 0, :, MID:PL])
            nc.sync.dma_start(xpf0[:, 1, MID:PL], xin_d[0, 1, :, MID:PL])
            load_xin(1, True)
            nc.sync.dma_start(xos[0][:], xout_d[0])
            load_xin(2, False)
            nc.sync.dma_start(xos[1][:], xout_d[1])
            load_xin(3, False)
            nc.sync.dma_start(xos[2][:], xout_d[2])
            nc.sync.dma_start(xos[3][:], xout_d[3])

            # ---- constants; zero guard planes for both pool buffers ----
            nc.vector.memset(sel[:], 0.0)
            for j in range(4):
                nc.vector.memset(sel[32 * j : 32 * j + 1, j, :], 1.0)
            nc.gpsimd.memset(wl[:], 0.0)
            nc.gpsimd.memset(wr[:], 0.0)
            for img in range(2):
                nc.scalar.memzero(hcols[img][:, :, :])
                nc.vector.memset(at9s[img][:, :, :, 0], 0.0)
                nc.vector.memset(at9s[img][:, :, :, 57], 0.0)

            def warm(n):
                for _ in range(n):
                    wp = ps_w.tile([128, 512], F32)
                    nc.tensor.matmul(
                        wp[:], wl[:], wr[:],
                        start=True, stop=True, skip_group_check=True,
                    )

            def emit_hcol(img):
                """2 independent SWDGE copies: dy=1,2 row-shifted replicas."""
                hf = hcols[img].rearrange("p r w -> p (r w)")
                nc.gpsimd.dma_start(hf[16:32, 0 : PL2 - 58], hf[0:16, 58:PL2])
                nc.gpsimd.dma_start(hf[32:48, 0 : PL2 - 116], hf[0:16, 116:PL2])

            def gen_conv1(img):
                """36 PE rounds; relu evacs into plane rows 1..58."""
                xpad = xpads[img]
                h1 = hcols[img]
                for s in range(2):
                    ps = ps_h.tile([128, 7, 56], F32)
                    rnd = 0
                    for k in range(KCH):
                        for t in range(9):
                            dy, dx = t // 3, t % 3
                            for j in range(4):
                                rs = 28 * s + j + dy
                                nc.tensor.matmul(
                                    ps[32 * j : 32 * j + 16],
                                    w1sb[:, k, t, :],
                                    xpad[:, k, rs : rs + 25 : 4, dx : dx + 56],
                                    start=(rnd == 0),
                                    stop=(rnd == 17),
                                    tile_position=(0, 32 * j),
                                    skip_group_check=True,
                                )
                            rnd += 1
                            if rnd == 18:
                                for j in range(4):
                                    r0 = 2 + 28 * s + j
                                    dst = h1[0:16, r0 : r0 + 25 : 4, 1:57]
                                    if j < 2:
                                        nc.scalar.activation(
                                            dst, ps[32 * j : 32 * j + 16],
                                            mybir.ActivationFunctionType.Relu,
                                        )
                                    else:
                                        nc.vector.tensor_scalar_max(
                                            dst, ps[32 * j : 32 * j + 16], 0.0
                                        )
                                if s == 1:
                                    # dy=1,2 row-shifted replicas for the
                                    # K=48 conv2 fold — DVE tensor_copy
                                    # runs in 4x bf16 mode (~0.9us each),
                                    # no DMA anywhere in the chain
                                    hf = h1.rearrange("p r w -> p (r w)")
                                    nc.vector.tensor_copy(
                                        hf[32:48, 0 : PL2 - 58],
                                        hf[0:16, 58:PL2],
                                    )
                                    nc.vector.tensor_copy(
                                        hf[64:80, 0 : PL2 - 116],
                                        hf[0:16, 116:PL2],
                                    )
                            yield

            def emit_boxsums(img, s):
                """Pool-engine rowsum3+colsum3 within the 9-row blocks."""
                at9, rsum, bb = at9s[img], rsums[img], bbs[img]
                if s == 0:
                    nc.gpsimd.memset(at9[0:1, 0, 0, :], 0.0)   # yp=0 pad row
                else:
                    nc.gpsimd.memset(at9[96:97, 1, 8, :], 0.0)  # yp=57 pad row
                # rowsum3 on Pool (it has headroom), colsum3 on DVE (2x)
                tmp9 = tmpp.tile([128, 9, 56], BF16, name="tmp9")
                nc.gpsimd.tensor_add(
                    tmp9[:], at9[:, s, :, 0:56], at9[:, s, :, 1:57]
                )
                nc.gpsimd.tensor_add(
                    rsum[:, s], tmp9[:], at9[:, s, :, 2:58]
                )
                tmp7 = tmpp.tile([128, 7, 56], BF16, name="tmp7")
                nc.gpsimd.tensor_add(
                    tmp7[:], rsum[:, s, 0:7], rsum[:, s, 1:8]
                )
                nc.gpsimd.tensor_add(bb[:, s], tmp7[:], rsum[:, s, 2:9])

            def gen_conv2(img):
                """6 PE rounds (K=48 dy-fold); sigmoid; Pool box sums."""
                hcol = hcols[img]
                at9 = at9s[img]
                for s in range(2):
                    ps = ps_a.tile([128, 9, 56], F32)
                    for dx in range(3):
                        for j in range(4):
                            b = 4 * s + j
                            nc.tensor.matmul(
                                ps[32 * j : 32 * j + 1],
                                w2sb[:, dx : dx + 1],
                                hcol[:, 7 * b : 7 * b + 9, dx : dx + 56],
                                start=(dx == 0), stop=(dx == 2),
                                tile_position=(0, 32 * j),
                                skip_group_check=True,
                            )
                        if dx < 2:
                            yield
                    nc.scalar.activation(
                        at9[:, s, :, 1:57], ps[:],
                        mybir.ActivationFunctionType.Sigmoid,
                    )
                    emit_boxsums(img, s)
                    yield

            def gen_conv2_direct(img):
                """18 direct 9-tap rounds — no hcol copies, no DMA deps.
                Used for the last image so the tail chain is DMA-free."""
                hcol = hcols[img]
                at9 = at9s[img]
                for s in range(2):
                    ps = ps_a.tile([128, 9, 56], F32)
                    rnd = 0
                    for dy in range(3):
                        for dx in range(3):
                            for j in range(4):
                                b = 4 * s + j
                                nc.tensor.matmul(
                                    ps[32 * j : 32 * j + 1],
                                    w2sb[:, 3 + rnd : 4 + rnd],
                                    hcol[:, 7 * b + dy : 7 * b + dy + 9,
                                         dx : dx + 56],
                                    start=(rnd == 0), stop=(rnd == 8),
                                    tile_position=(0, 32 * j),
                                    skip_group_check=True,
                                )
                            rnd += 1
                            if rnd == 9:
                                nc.scalar.activation(
                                    at9[:, s, :, 1:57], ps[:],
                                    mybir.ActivationFunctionType.Sigmoid,
                                )
                                emit_boxsums(img, s)
                            yield

            def gen_box(img):
                """8 blocks: selector-matmul broadcast, evac, bf16 muls."""
                bb = bbs[img]
                xo = xos[img]
                ab = abp.tile([128, 56, 56], BF16)
                abf = ab.rearrange("p r w -> p (r w)")
                ot = otp.tile([128, KCH, HW], BF16)

                def halfdone(h):
                    s0, s1 = (0, 1568) if h == 0 else (1568, HW)
                    for k in range(KCH):
                        nc.vector.tensor_mul(
                            ot[:, k, s0:s1], xo[:, k, s0:s1], abf[:, s0:s1]
                        )
                    nc.scalar.dma_start(
                        out_d[img, :, :, s0:s1], ot[:, :, s0:s1]
                    )

                for b in range(8):
                    s, j = b // 4, b % 4
                    psb = ps_b.tile([128, 7, 56], F32)
                    nc.tensor.matmul(
                        psb[:], sel[:, j, :], bb[:, s, :, :],
                        start=True, stop=True,
                    )
                    dst = ab[:, 7 * b : 7 * b + 7, :]
                    if b % 4 != 3:
                        nc.scalar.activation(
                            dst, psb[:], mybir.ActivationFunctionType.Copy
                        )
                    else:
                        nc.vector.tensor_copy(dst, psb[:])
                    if b == 3:
                        halfdone(0)
                    yield
                halfdone(1)
                yield

            def run(gen, n):
                for _ in range(n):
                    next(gen, None)

            c1 = [gen_conv1(i) for i in range(BL)]
            c2 = [gen_conv2(i) for i in range(BL)]
            bx = [gen_box(i) for i in range(BL)]

            def block(i):
                # c1 r0-23 solo, conv2(i-1) 6 rounds at r24-29 (1:1),
                # box(i-2) at r30-35 (1:1) + small burst.
                if i >= 3:
                    run(bx[i - 3], 1)    # deferred second-half muls+store
                run(c1[i], 24)
                for _ in range(6):
                    run(c2[i - 1], 1)
                    run(c1[i], 1)
                for _ in range(6):
                    run(bx[i - 2], 1)
                    run(c1[i], 1)
                run(bx[i - 2], 2)

            warm(3)
            run(c1[0], 36)
            run(c1[1], 24)
            for _ in range(6):
                run(c2[0], 1)
                run(c1[1], 1)
            run(c1[1], 6)
            block(2)
            block(3)
            # tail: finish box(1); box(2) first (its bb is long ready),
            # then conv2(3) once the DVE replicas land, then box(3)
            run(bx[1], 1)
            run(bx[2], 4)
            for _ in range(3):
                run(c2[3], 2)
                run(bx[2], 1)
            run(bx[2], 1)
            warm(8)
            run(bx[3], 4)
            warm(2)
            run(bx[3], 4)
            run(bx[2], 1)
            run(bx[3], 1)

    nc.compile()
    return nc


def _prep_shards(x_in, x_out, w1, w2):
    bf16 = ml_dtypes.bfloat16
    fp8 = ml_dtypes.float8_e4m3
    # w1t[c, k, t, m] = w1[m, 128k + c, dy, dx],  t = 3*dy + dx
    w1t = np.ascontiguousarray(
        w1.reshape(16, KCH, 128, 9).transpose(2, 1, 3, 0)
    ).astype(fp8)
    # w2t cols 0-2:  dy-fold (replicas at 32-aligned partition bases)
    #   w2t[32*dy + m, dx] = w2[0, m, dy, dx]
    # w2t cols 3-11: direct taps  w2t[m, 3 + 3*dy + dx] = w2[0, m, dy, dx]
    w2t = np.zeros((128, 12), dtype=bf16)
    for dy in range(3):
        w2t[32 * dy : 32 * dy + 16, 0:3] = w2[0, :, dy, :].astype(bf16)
    w2t[0:16, 3:12] = w2[0].reshape(16, 9).astype(bf16)
    xi = np.zeros((NCORES, BL, KCH, 128, HP, HP), dtype=fp8)
    xi[..., 1 : 1 + H, 1 : 1 + W] = (
        x_in.reshape(NCORES, BL, KCH, 128, H, W).astype(fp8)
    )
    xi = xi.reshape(NCORES, BL, KCH, 128, PL)
    # xout[img, c_partition, k, hw]
    xo = np.ascontiguousarray(
        x_out.reshape(NCORES, BL, KCH, 128, HW).transpose(0, 1, 3, 2, 4)
    ).astype(bf16)
    return [
        {
            "xin": np.ascontiguousarray(xi[i]),
            "xout": xo[i],
            "w1t": w1t,
            "w2t": w2t,
        }
        for i in range(NCORES)
    ]


def _run(in_maps, trace=False):
    if "nc" not in _cache:
        _cache["nc"] = _build()
    return run_bass_kernel_spmd(
        _cache["nc"], in_maps, core_ids=list(range(NCORES)), trace=trace
    )


def kernel(x_in, x_out, w1, w2, _trace=False):
    in_maps = _prep_shards(
        np.asarray(x_in, dtype=np.float32),
        np.asarray(x_out, dtype=np.float32),
        np.asarray(w1, dtype=np.float32),
        np.asarray(w2, dtype=np.float32),
    )
    res = _run(in_maps, trace=_trace)
    # out[img, c_partition, k, hw] bf16 -> [B, C, H, W] fp32
    out = np.stack([res.results[i]["out"] for i in range(NCORES)])
    kernel.last_exec_time_ns = res.exec_time_ns
    out = out.astype(np.float32).transpose(0, 1, 3, 2, 4)
    return out.reshape(B, C, H, W)


# revision 69
# speedup vs baseline: 1.0979x; 1.0425x over previous
"""ALayer kernel for 8 TRN2 NeuronCores — pure data parallel over batch.

Per-core shard: 4 images of [256, 56, 56].
  h  = relu(conv3x3(x_in, w1))      # 256 -> 16 ch
  A  = sigmoid(conv3x3(h, w2))      # 16 -> 1 ch
  out = x_out * box3x3(A)           # broadcast over 256 ch

v20 design — no DMA in the mid-pipeline dep chains (TRN2 SDMA serves
SBUF->SBUF SWDGE transfers only when bulk HBM queues are empty, so any
DMA-built intermediate starves; measured 7-20us stalls):
  conv1: v6's column-tiled rounds (4 concurrent 32-col strips, fp8,
         M=16, N=392; 36 rounds/image).  Relu evacs write the h plane
         = hcol[0:16] rows 1..58 of a 61-row guard-padded tile.
  conv2: K=48 dy-fold: hcol[16*dy+m] = plane shifted 58*dy (2 paced
         SWDGE copies, images 0-2; dx shifts come free via the rhs AP).
         3 col-tiled rounds x 2 supergroups, OVERLAPPING 9-row blocks
         (psum 504 <= 512).  Image 3 uses the direct 9-tap form (18
         rounds, zero DMA deps) so the tail has no DMA latency at all.
  box:   box3x3 = rowsum3+colsum3 computed per-partition on the Pool
         engine from the 9-row sigmoid blocks (each block carries its
         halo rows, so no cross-partition traffic), then broadcast to
         128 partitions with a K=128 selector matmul; evac to bf16 ab
         (ACT/DVE), bf16 muls (DVE 2x) in <=1us chunks, stores on the
         ACT HWDGE ring.
  Loads ride the SP HWDGE ring, paced by pool-reuse WAR gates
         (xpad/xo bufs=2) + emission order so the ring has idle windows
         exactly when the hcol copies fire.  Zero-padded-K matmuls keep
         the HAM activity monitor seeing a busy array (clock 8/8).
"""

import numpy as np
import ml_dtypes

import concourse.bass as bass
import concourse.tile as tile
import concourse.mybir as mybir
from concourse import bacc
from concourse.bass_utils import run_bass_kernel_spmd

BF16 = mybir.dt.bfloat16
FP8 = mybir.dt.float8e4
F32 = mybir.dt.float32

B, C, H, W = 32, 256, 56, 56
NCORES = 8
BL = B // NCORES          # images per core
KCH = 2                   # 256 = 2 chunks of 128
HP = H + 2                # padded plane side (58)
HT = 61                   # hcol tile rows: top guard + plane + bottom slack
HW = H * W                # 3136
PL = HP * HP              # 3364
PL2 = HT * HP             # 3538

_cache = {}


def _build():
    nc = bacc.Bacc("TRN2", target_bir_lowering=False, debug=False)

    xin_d = nc.dram_tensor("xin", [BL, KCH, 128, PL], FP8, kind="ExternalInput").ap()
    xout_d = nc.dram_tensor("xout", [BL, 128, KCH, HW], BF16, kind="ExternalInput").ap()
    w1_d = nc.dram_tensor("w1t", [128, KCH, 9, 16], FP8, kind="ExternalInput").ap()
    w2_d = nc.dram_tensor("w2t", [128, 12], BF16, kind="ExternalInput").ap()
    out_d = nc.dram_tensor("out", [BL, 128, KCH, HW], BF16, kind="ExternalOutput").ap()

    with tile.TileContext(nc) as tc:
        with (
            tc.tile_pool(name="const", bufs=1) as constp,
            tc.tile_pool(name="xpad", bufs=4) as xpadp,
            tc.tile_pool(name="hcol", bufs=2) as hcolp,
            tc.tile_pool(name="at9", bufs=2) as atp,
            tc.tile_pool(name="sums", bufs=2) as sump,
            tc.tile_pool(name="tmps", bufs=4) as tmpp,
            tc.tile_pool(name="ab", bufs=2) as abp,
            tc.tile_pool(name="xo", bufs=4) as xop,
            tc.tile_pool(name="ot", bufs=2) as otp,
            tc.tile_pool(name="ps_h", bufs=2, space="PSUM") as ps_h,
            tc.tile_pool(name="ps_a", bufs=2, space="PSUM") as ps_a,
            tc.tile_pool(name="ps_b", bufs=3, space="PSUM") as ps_b,
            tc.tile_pool(name="ps_w", bufs=1, space="PSUM") as ps_w,
        ):
            w1sb = constp.tile([128, KCH, 9, 16], FP8)
            w2sb = constp.tile([128, 12], BF16)
            sel = constp.tile([128, 4, 128], BF16)
            wl = constp.tile([128, 128], FP8)
            wr = constp.tile([128, 512], FP8)

            xpads = [
                xpadp.tile([128, KCH, HP, HP], FP8, name="xpad")
                for _ in range(BL)
            ]
            xos = [xop.tile([128, KCH, HW], BF16, name="xo") for _ in range(BL)]
            hcols = [
                hcolp.tile([128, HT, HP], BF16, name="hcol") for _ in range(BL)
            ]
            at9s = [
                atp.tile([128, 2, 9, HP], BF16, name="at9") for _ in range(BL)
            ]
            rsums = [
                sump.tile([128, 2, 9, 56], BF16, name="rsum") for _ in range(BL)
            ]
            bbs = [
                sump.tile([128, 2, 7, 56], BF16, name="bb") for _ in range(BL)
            ]
            MID = 30 * HP

            def load_xin(img, split):
                xpf = xpads[img].rearrange("p k r w -> p k (r w)")
                if split:
                    for k in range(KCH):
                        nc.sync.dma_start(xpf[:, k, 0:MID], xin_d[img, k, :, 0:MID])
                    for k in range(KCH):
                        nc.sync.dma_start(xpf[:, k, MID:PL], xin_d[img, k, :, MID:PL])
                else:
                    for k in range(KCH):
                        nc.sync.dma_start(xpf[:, k, :], xin_d[img, k, :, :])

            # ---- loads: head batch, then WAR-gated stragglers ----
            xpf0 = xpads[0].rearrange("p k r w -> p k (r w)")
            nc.sync.dma_start(xpf0[:, 0, 0:MID], xin_d[0, 0, :, 0:MID])
            nc.sync.dma_start(w1sb[:], w1_d[:])
            nc.sync.dma_start(w2sb[:], w2_d[:])
            nc.sync.dma_start(xpf0[:, 1, 0:MID], xin_d[0, 1, :, 0:MID])
            nc.sync.dma_start(xpf0[:, 0, MID:PL], xin_d[0, 0, :, MID:PL])
            nc.sync.dma_start(xpf0[:, 1, MID:PL], xin_d[0, 1, :, MID:PL])
            load_xin(1, True)
            nc.sync.dma_start(xos[0][:], xout_d[0])
            load_xin(2, False)
            nc.sync.dma_start(xos[1][:], xout_d[1])
            load_xin(3, False)
            nc.sync.dma_start(xos[2][:], xout_d[2])
            nc.sync.dma_start(xos[3][:], xout_d[3])

            # ---- constants; zero guard planes for both pool buffers ----
            nc.vector.memset(sel[:], 0.0)
            for j in range(4):
                nc.vector.memset(sel[32 * j : 32 * j + 1, j, :], 1.0)
            nc.gpsimd.memset(wl[:], 0.0)
            nc.gpsimd.memset(wr[:], 0.0)
            for img in range(2):
                nc.scalar.memzero(hcols[img][:, :, :])
                nc.vector.memset(at9s[img][:, :, :, 0], 0.0)
                nc.vector.memset(at9s[img][:, :, :, 57], 0.0)

            def warm(n):
                for _ in range(n):
                    wp = ps_w.tile([128, 512], F32)
                    nc.tensor.matmul(
                        wp[:], wl[:], wr[:],
                        start=True, stop=True, skip_group_check=True,
                    )

            def emit_hcol(img):
                """2 independent SWDGE copies: dy=1,2 row-shifted replicas."""
                hf = hcols[img].rearrange("p r w -> p (r w)")
                nc.gpsimd.dma_start(hf[16:32, 0 : PL2 - 58], hf[0:16, 58:PL2])
                nc.gpsimd.dma_start(hf[32:48, 0 : PL2 - 116], hf[0:16, 116:PL2])

            def gen_conv1(img):
                """36 PE rounds; relu evacs into plane rows 1..58."""
                xpad = xpads[img]
                h1 = hcols[img]
                for s in range(2):
                    ps = ps_h.tile([128, 7, 56], F32)
                    rnd = 0
                    for k in range(KCH):
                        for t in range(9):
                            dy, dx = t // 3, t % 3
                            for j in range(4):
                                rs = 28 * s + j + dy
                                nc.tensor.matmul(
                                    ps[32 * j : 32 * j + 16],
                                    w1sb[:, k, t, :],
                                    xpad[:, k, rs : rs + 25 : 4, dx : dx + 56],
                                    start=(rnd == 0),
                                    stop=(rnd == 17),
                                    tile_position=(0, 32 * j),
                                    skip_group_check=True,
                                )
                            rnd += 1
                            if rnd == 18:
                                for j in range(4):
                                    r0 = 2 + 28 * s + j
                                    dst = h1[0:16, r0 : r0 + 25 : 4, 1:57]
                                    if j < 2:
                                        nc.scalar.activation(
                                            dst, ps[32 * j : 32 * j + 16],
                                            mybir.ActivationFunctionType.Relu,
                                        )
                                    else:
                                        nc.vector.tensor_scalar_max(
                                            dst, ps[32 * j : 32 * j + 16], 0.0
                                        )
                            yield

            def emit_boxsums(img, s):
                """Pool-engine rowsum3+colsum3 within the 9-row blocks."""
                at9, rsum, bb = at9s[img], rsums[img], bbs[img]
                if s == 0:
                    nc.gpsimd.memset(at9[0:1, 0, 0, :], 0.0)   # yp=0 pad row
                else:
                    nc.gpsimd.memset(at9[96:97, 1, 8, :], 0.0)  # yp=57 pad row
                # rowsum3 on DVE (2x bf16 ~420ns/op), colsum3 on Pool
                tmp9 = tmpp.tile([128, 9, 56], BF16, name="tmp9")
                nc.vector.tensor_add(
                    tmp9[:], at9[:, s, :, 0:56], at9[:, s, :, 1:57]
                )
                nc.vector.tensor_add(
                    rsum[:, s], tmp9[:], at9[:, s, :, 2:58]
                )
                tmp7 = tmpp.tile([128, 7, 56], BF16, name="tmp7")
                nc.gpsimd.tensor_add(
                    tmp7[:], rsum[:, s, 0:7], rsum[:, s, 1:8]
                )
                nc.gpsimd.tensor_add(bb[:, s], tmp7[:], rsum[:, s, 2:9])

            def gen_conv2(img):
                """6 PE rounds (K=48 dy-fold); sigmoid; Pool box sums."""
                hcol = hcols[img]
                at9 = at9s[img]
                for s in range(2):
                    ps = ps_a.tile([128, 9, 56], F32)
                    for dx in range(3):
                        for j in range(4):
                            b = 4 * s + j
                            nc.tensor.matmul(
                                ps[32 * j : 32 * j + 1],
                                w2sb[:, dx : dx + 1],
                                hcol[:, 7 * b : 7 * b + 9, dx : dx + 56],
                                start=(dx == 0), stop=(dx == 2),
                                tile_position=(0, 32 * j),
                                skip_group_check=True,
                            )
                        if dx < 2:
                            yield
                    nc.scalar.activation(
                        at9[:, s, :, 1:57], ps[:],
                        mybir.ActivationFunctionType.Sigmoid,
                    )
                    emit_boxsums(img, s)
                    yield

            def gen_conv2_direct(img):
                """18 direct 9-tap rounds — no hcol copies, no DMA deps.
                Used for the last image so the tail chain is DMA-free."""
                hcol = hcols[img]
                at9 = at9s[img]
                for s in range(2):
                    ps = ps_a.tile([128, 9, 56], F32)
                    rnd = 0
                    for dy in range(3):
                        for dx in range(3):
                            for j in range(4):
                                b = 4 * s + j
                                nc.tensor.matmul(
                                    ps[32 * j : 32 * j + 1],
                                    w2sb[:, 3 + rnd : 4 + rnd],
                                    hcol[:, 7 * b + dy : 7 * b + dy + 9,
                                         dx : dx + 56],
                                    start=(rnd == 0), stop=(rnd == 8),
                                    tile_position=(0, 32 * j),
                                    skip_group_check=True,
                                )
                            rnd += 1
                            if rnd == 9:
                                nc.scalar.activation(
                                    at9[:, s, :, 1:57], ps[:],
                                    mybir.ActivationFunctionType.Sigmoid,
                                )
                                emit_boxsums(img, s)
                            yield

            def gen_box(img):
                """8 blocks: selector-matmul broadcast, evac, bf16 muls."""
                bb = bbs[img]
                xo = xos[img]
                ab = abp.tile([128, 56, 56], BF16)
                abf = ab.rearrange("p r w -> p (r w)")
                ot = otp.tile([128, KCH, HW], BF16)

                def halfdone(h):
                    s0, s1 = (0, 1568) if h == 0 else (1568, HW)
                    for k in range(KCH):
                        nc.vector.tensor_mul(
                            ot[:, k, s0:s1], xo[:, k, s0:s1], abf[:, s0:s1]
                        )
                    nc.scalar.dma_start(
                        out_d[img, :, :, s0:s1], ot[:, :, s0:s1]
                    )

                for b in range(8):
                    s, j = b // 4, b % 4
                    psb = ps_b.tile([128, 7, 56], F32)
                    nc.tensor.matmul(
                        psb[:], sel[:, j, :], bb[:, s, :, :],
                        start=True, stop=True,
                    )
                    dst = ab[:, 7 * b : 7 * b + 7, :]
                    if b % 2 == 0:
                        nc.scalar.activation(
                            dst, psb[:], mybir.ActivationFunctionType.Copy
                        )
                    else:
                        nc.vector.tensor_copy(dst, psb[:])
                    if b == 3:
                        halfdone(0)
                    yield
                halfdone(1)
                yield

            def run(gen, n):
                for _ in range(n):
                    next(gen, None)

            c1 = [gen_conv1(i) for i in range(BL)]
            c2 = [gen_conv2_direct(i) for i in range(BL)]
            bx = [gen_box(i) for i in range(BL)]

            def block(i):
                # c1 r0-23 solo, conv2(i-1) 18 rounds at r24-32 (2:1),
                # box(i-2) at r33-35 + burst.
                if i >= 3:
                    run(bx[i - 3], 1)    # deferred second-half muls+store
                run(c1[i], 24)
                for _ in range(9):
                    run(c2[i - 1], 2)
                    run(c1[i], 1)
                for _ in range(3):
                    run(bx[i - 2], 1)
                    run(c1[i], 1)
                run(bx[i - 2], 5)

            warm(3)
            run(c1[0], 36)
            run(c1[1], 24)
            for _ in range(9):
                run(c2[0], 2)
                run(c1[1], 1)
            run(c1[1], 3)
            block(2)
            block(3)
            # tail: finish box(1); conv2(3); box(2); box(3)
            run(bx[1], 1)
            warm(3)
            for _ in range(6):
                run(c2[3], 3)
                run(bx[2], 1)
            run(bx[2], 2)
            warm(6)
            run(bx[3], 8)
            run(bx[2], 1)
            run(bx[3], 1)

    nc.compile()
    return nc


def _prep_shards(x_in, x_out, w1, w2):
    bf16 = ml_dtypes.bfloat16
    fp8 = ml_dtypes.float8_e4m3
    # w1t[c, k, t, m] = w1[m, 128k + c, dy, dx],  t = 3*dy + dx
    w1t = np.ascontiguousarray(
        w1.reshape(16, KCH, 128, 9).transpose(2, 1, 3, 0)
    ).astype(fp8)
    # w2t cols 0-2:  dy-fold (replicas at 32-aligned partition bases)
    #   w2t[32*dy + m, dx] = w2[0, m, dy, dx]
    # w2t cols 3-11: direct taps  w2t[m, 3 + 3*dy + dx] = w2[0, m, dy, dx]
    w2t = np.zeros((128, 12), dtype=bf16)
    for dy in range(3):
        w2t[32 * dy : 32 * dy + 16, 0:3] = w2[0, :, dy, :].astype(bf16)
    w2t[0:16, 3:12] = w2[0].reshape(16, 9).astype(bf16)
    xi = np.zeros((NCORES, BL, KCH, 128, HP, HP), dtype=fp8)
    xi[..., 1 : 1 + H, 1 : 1 + W] = (
        x_in.reshape(NCORES, BL, KCH, 128, H, W).astype(fp8)
    )
    xi = xi.reshape(NCORES, BL, KCH, 128, PL)
    # xout[img, c_partition, k, hw]
    xo = np.ascontiguousarray(
        x_out.reshape(NCORES, BL, KCH, 128, HW).transpose(0, 1, 3, 2, 4)
    ).astype(bf16)
    return [
        {
            "xin": np.ascontiguousarray(xi[i]),
            "xout": xo[i],
            "w1t": w1t,
            "w2t": w2t,
        }
        for i in range(NCORES)
    ]


def _run(in_maps, trace=False):
    if "nc" not in _cache:
        _cache["nc"] = _build()
    return run_bass_kernel_spmd(
        _cache["nc"], in_maps, core_ids=list(range(NCORES)), trace=trace
    )


def kernel(x_in, x_out, w1, w2, _trace=False):
    in_maps = _prep_shards(
        np.asarray(x_in, dtype=np.float32),
        np.asarray(x_out, dtype=np.float32),
        np.asarray(w1, dtype=np.float32),
        np.asarray(w2, dtype=np.float32),
    )
    res = _run(in_maps, trace=_trace)
    # out[img, c_partition, k, hw] bf16 -> [B, C, H, W] fp32
    out = np.stack([res.results[i]["out"] for i in range(NCORES)])
    kernel.last_exec_time_ns = res.exec_time_ns
    out = out.astype(np.float32).transpose(0, 1, 3, 2, 4)
    return out.reshape(B, C, H, W)
